# revision 6
# baseline (speedup 1.0000x reference)
"""Overlapping-chunk extraction for Trainium2: 3 pure DMAs per core.

out[b, j, c, f] = x[b, 125*j + c, f] for j<255, c<250 — 255 half-
overlapping 250-frame chunks of a (16, 32000, 64) fp32 signal.  Batch is
sharded across 8 cores (2 samples per core).

Structural insight: split the signal into 125-frame HALF-chunks H_m
(8000 fp32 each; an even chunk 2k = [H_2k H_2k+1], an odd chunk 2k+1 =
[H_2k+1 H_2k+2]).  The flat output is then

    y = H_0 [H_1 H_1] [H_2 H_2] ... [H_510 H_510] H_511

i.e. every half-chunk written twice to ADJACENT 32000-byte slots — pure
data movement, no compute.  With a per-sample padded output layout
[8000 pre | 255*16000 data | 8000 post] = 4_096_000 fp32, the mapping is
uniform across the whole core (4_096_000 = 64 * 64_000 keeps partition
bases aligned across the 2 samples):

    y_pad[16000*m + 0    .. +8000) = H_m    (odd-chunk second halves;
                                             m=0,256 land in the pre-pads)
    y_pad[16000*m + 8000 .. +8000) = H_m    (even-chunk first halves;
                                             m=255,511 land in the post-pads)

Kernel per core (SBUF tile bufAll [128, 32000] fp32, partition k = halves
4k..4k+3 = x elements [32000k, 32000k+32000)):

    in : x  -> bufAll   contiguous 16.4MB read, full-flat SBUF side
    c0 : bufAll -> y    [[64000,128],[16000,4],[1,8000]] @ 0     (16.4MB)
    c1 : bufAll -> y    [[64000,128],[16000,4],[1,8000]] @ 8000  (16.4MB)

49.15MB HBM traffic/core = the information-theoretic minimum (input read
once, output written once); both phases measured at the ~358 GB/s per-NC
HBM cap, so the kernel sits ~7% above the 137.3us bus floor.

Ring assignment: in and c0 ride the SWDGE ring (gpsimd); c1 rides the
HWDGE ring (sync/SP — measured marginally ahead of the ACT HWDGE ring
in repeated A/B).  in->c0 needs no semaphore: SBUF partition p is
hard-wired to one SBUF AXI port = one SDMA engine (port =
((p>>2)&7)<<1 | ((p>>6)&1); AWS-confirmed fixed table), and one ring is
FIFO per engine, so in's write of bufAll[p] precedes c0's read of it by
ring order alone — engines flow from the read phase into the write phase
with no receipt/wake bubble.  c1 is on a different ring, so it IS gated
on in's completion semaphore; it then drains concurrently with c0
(engines round-robin the two rings at packet granularity), overlapping
the two writes' emission and completion latencies.  Measured (8-core
SPMD, repeat-differencing, median): ~146.0us/iter vs 207.5us for the
previous matmul-shift kernel (two-ring outs beat single-ring by ~1.5us).
"""

import numpy as np

import concourse.bass as bass
import concourse.mybir as mybir
from concourse.bass_utils import run_bass_kernel_spmd

# Problem shape (hardcoded per contract)
B, T, F = 16, 32000, 64
N_CORES = 8
S = B // N_CORES              # samples per core = 2
SAMPLE_IN = T * F             # 2_048_000 fp32 per input sample
HALF = 8000                   # fp32 per half-chunk (125 frames * 64 filt)
NOV = 255                     # overlapped output chunks per sample
CHUNK = 250                   # frames per chunk
SAMPLE_OUT = NOV * CHUNK * F              # 4_080_000 fp32
SAMPLE_PAD = SAMPLE_OUT + 2 * HALF        # 4_096_000 fp32 (pre+post pad)
W = 32000                     # bufAll cols = 4 half-chunks per partition

F32 = mybir.dt.float32
_NC_CACHE = {}


def build_module(repeat=1, name="chunkop"):
    """Build the kernel program; `repeat` chains the kernel body R times
    (phase-locked via semaphores) for HW timing via differencing."""
    nc = bass.Bass(trn_type="TRN2", name=name)
    x = nc.dram_tensor("x", [S * SAMPLE_IN], F32, kind="ExternalInput")
    y = nc.dram_tensor("y", [S * SAMPLE_PAD], F32, kind="ExternalOutput")
    x_t = x[:].tensor
    y_t = y[:].tensor

    from contextlib import ExitStack

    with ExitStack() as ctx:
        buf = ctx.enter_context(nc.sbuf_tensor("bufAll", [128, W], F32))
        s_in = ctx.enter_context(nc.semaphore("s_in"))
        s_out = ctx.enter_context(nc.semaphore("s_out"))
        blk = ctx.enter_context(nc.Block())

        @blk.gpsimd
        def _(g):
            with nc.allow_non_contiguous_dma(reason="strided half-chunk dup"):
                for r in range(repeat):
                    if r > 0:
                        # bufAll reuse across reps: phase-lock on the
                        # previous rep's outs (keeps HBM read and write
                        # phases separated; free-running measured slower)
                        g.wait_ge(s_out, 32 * r)
                    src = bass.AP(x_t, 0, [[W, 128], [1, W]])
                    g.dma_start(buf[:, :], src).then_inc(s_in, 16)
                    # c0 needs no wait: same-ring per-engine FIFO plus the
                    # fixed partition->port wiring already orders its read
                    # of bufAll[p] after in's write of it
                    dst = bass.AP(y_t, 0,
                                  [[2 * W, 128], [2 * HALF, 4], [1, HALF]])
                    g.dma_start(dst, buf[:, :]).then_inc(s_out, 16)
                g.wait_ge(s_out, 32 * repeat)

        @blk.sync
        def _(sc):
            with nc.allow_non_contiguous_dma(reason="strided half-chunk dup"):
                for r in range(repeat):
                    # cross-ring data dependency: gate c1 on in's completion
                    sc.wait_ge(s_in, 16 * (r + 1))
                    dst = bass.AP(y_t, HALF,
                                  [[2 * W, 128], [2 * HALF, 4], [1, HALF]])
                    sc.dma_start(dst, buf[:, :]).then_inc(s_out, 16)
                sc.wait_ge(s_out, 32 * repeat)

    return nc


def get_module():
    if "nc" not in _NC_CACHE:
        _NC_CACHE["nc"] = build_module()
    return _NC_CACHE["nc"]


def kernel(x):
    x = np.ascontiguousarray(np.asarray(x), dtype=np.float32)
    assert x.shape == (B, T, F), x.shape
    nc = get_module()
    in_maps = [{"x": x[i * S:(i + 1) * S].reshape(-1)} for i in range(N_CORES)]
    res = run_bass_kernel_spmd(nc, in_maps, core_ids=list(range(N_CORES)))
    outs = [
        r["y"].reshape(S, SAMPLE_PAD)[:, HALF:HALF + SAMPLE_OUT]
        .reshape(S, NOV, CHUNK, F)
        for r in res.results
    ]
    return np.concatenate(outs, axis=0)
